# revision 21
# baseline (speedup 1.0000x reference)
"""Trainium2 Bass kernel for nn_BSplineActivation.

y(x) = sum_j w_j B_j(x) for a degree-3 B-spline on a (nearly) uniform
knot grid (1024 fp32 knots on [-pi, pi]). Restricted to knot interval i,
y is an exact cubic polynomial; only the 4 coefficients (expanded around
m_i = knots32[i]) depend on the weights.

Strategy (vs. the old per-column indirect-DMA gather, which serialized
32768 tiny descriptors per core at ~0.5us each):
  * Host (weights-only preprocessing): per-interval cubic coefficients
    (b0..b3 in u = x - m_i) + m_i, computed in float64 from the uniform
    cardinal B-spline form, stored as a [1024, 64] f32 table (rows padded
    to 256 B -- the dma_gather granularity).
  * Device: interval index i as int16 (computed twice: once in the
    "wrapped" layout dma_gather wants for its index list, conceptually);
    ONE batched gpsimd.dma_gather pulls all 32768 rows per core across
    the 16 DMA engines; Horner in u; mask outside [knot0, knot_last).
  * Data parallel over 8 NeuronCores; table replicated.

Layouts per core (32768 points, j = point index within the core):
  dense  x_d [128, 256] : point j at [j % 128, j // 128]  (matches
         dma_gather output order: out[p, c, :] = row(idx[c*128+p]))
  wrap   xw_d [128, 2048]: point j at [16g + (j % 16), j // 16] for all
         g (dma_gather reads its index list wrapped over 16 partitions;
         replicated across the 8 gpsimd cores' partition groups)
"""
import sys

sys.path.insert(0, "/opt/trn_rl_repo")

import numpy as np

import concourse.bacc as bacc
import concourse.mybir as mybir
from concourse.bass_utils import run_bass_kernel_spmd
from concourse.library_config import mlp as _mlp_lib

P, F = 128, 256          # dense layout: 128 partitions x 256 points
NCORES = 8
NPTS_C = P * F           # 32768 points per core
NPTS = NCORES * NPTS_C   # 262144
NUM_KNOTS = 1024
DEGREE = 3
NW = NUM_KNOTS - DEGREE - 1  # 1020 weights
NI = NUM_KNOTS - 1           # 1023 intervals
NROWS = 1024                 # table rows (>= NI, pow2)
ELEM = 64                    # f32 per table row = 256 B (dma_gather min)
WRAP_F = NPTS_C // 16        # 2048
NCHUNK = 16                  # gathers per tile (swdge FIFO is 128 entries)
CH_IDX = NPTS_C // NCHUNK    # 2048 indices per gather
CH_COLS = CH_IDX // P        # 16 output columns per gather
CH_W = CH_IDX // 16          # 128 wrapped index slots per gather

f32 = mybir.dt.float32
i16 = mybir.dt.int16
AL = mybir.AluOpType

_KNOTS32 = np.linspace(-np.pi, np.pi, NUM_KNOTS).astype(np.float32)
_T0 = float(_KNOTS32[0])
_TLAST = float(_KNOTS32[-1])
# z = x * INV_H + CB maps x to the (approximate) interval coordinate.
_H64 = (float(_KNOTS32[-1]) - float(_KNOTS32[0])) / float(NI)
_INV_H = float(np.float32(1.0 / _H64))
_CB = float(np.float32(-float(_KNOTS32[0]) / _H64))
_CLMAX = 1022.9995


def _build_table(weights: np.ndarray) -> np.ndarray:
    """[1024, 64] f32: row i = (b0, b1, b2, b3, m_i, 0...) with
    y(x) = ((b3*u + b2)*u + b1)*u + b0, u = x - m_i, for x in interval i.

    Exact (f64) uniform cardinal B-spline: on interval i with
    s = (x - t0)/h - i, y = sum_k wpad[i+3-k] * piece_k(s)."""
    w64 = np.asarray(weights, dtype=np.float64)
    wpad = np.zeros(NI + 3, dtype=np.float64)
    wpad[3 : 3 + NW] = w64
    # piece_k(s) coefficients (1, s, s^2, s^3), k = 0..3
    C = np.array(
        [[0.0, 0.0, 0.0, 1.0],
         [1.0, 3.0, 3.0, -3.0],
         [4.0, 0.0, -6.0, 3.0],
         [1.0, -3.0, 3.0, -1.0]]) / 6.0
    idx = np.arange(NI)
    W4 = np.stack([wpad[idx + 3], wpad[idx + 2], wpad[idx + 1], wpad[idx]], 1)
    A = W4 @ C                                  # [1023, 4]: a_e in s-units
    t64 = _KNOTS32.astype(np.float64)
    h = (t64[-1] - t64[0]) / NI
    m = t64[idx]                                # expansion points (f32 knots)
    sig = (t64[0] + idx * h - m) / h            # s at u=0 (fp32 knot jitter)
    a0, a1, a2, a3 = A.T
    b0 = ((a3 * sig + a2) * sig + a1) * sig + a0
    b1 = ((3.0 * a3 * sig + 2.0 * a2) * sig + a1) / h
    b2 = (3.0 * a3 * sig + a2) / (h * h)
    b3 = a3 / (h * h * h)
    tab = np.zeros((NROWS, ELEM), dtype=np.float32)
    tab[:NI, 0] = b0
    tab[:NI, 1] = b1
    tab[:NI, 2] = b2
    tab[:NI, 3] = b3
    tab[:NI, 4] = m
    return tab


_NC_CACHE = {}


def _build_nc(reps: int = 1):
    """reps > 1 repeats the full compute body (for device-time estimation)."""
    nc = bacc.Bacc("TRN2", target_bir_lowering=False, debug=False,
                   num_devices=NCORES, num_swdge_queues=4)
    x_d = nc.dram_tensor("x", [P, F], f32, kind="ExternalInput")
    xw_d = nc.dram_tensor("xw", [P, WRAP_F], f32, kind="ExternalInput")
    tab_ds = [nc.dram_tensor(f"tab{q}", [NROWS, ELEM], f32,
                             kind="ExternalInput") for q in range(4)]
    y_d = nc.dram_tensor("y", [P, F], f32, kind="ExternalOutput")

    xt = nc.alloc_sbuf_tensor("xt", [P, F], f32)
    xwt = nc.alloc_sbuf_tensor("xwt", [P, WRAP_F], f32)
    zwt = nc.alloc_sbuf_tensor("zwt", [P, WRAP_F], f32)
    idxt = nc.alloc_sbuf_tensor("idxt", [P, WRAP_F], i16)
    gv = nc.alloc_sbuf_tensor("gv", [P, F, ELEM], f32)
    ut = nc.alloc_sbuf_tensor("ut", [P, F], f32)
    acct = nc.alloc_sbuf_tensor("acct", [P, F], f32)
    m1t = nc.alloc_sbuf_tensor("m1t", [P, F], f32)
    yt = nc.alloc_sbuf_tensor("yt", [P, F], f32)

    io = nc.alloc_semaphore("io")     # input/output DMA completions (+16)
    cs = nc.alloc_semaphore("cs")     # DVE milestones (idx ready / y ready)
    gs = nc.alloc_semaphore("gs")     # gather DMA completions (+16)
    vs = nc.alloc_semaphore("vs")     # DVE instruction counter (RAW chaining)

    def col(e):  # [P, F] view of gather payload element e
        return gv[:, :, e : e + 1].rearrange("p f e -> p (f e)")

    with nc.Block() as block:

        @block.gpsimd
        def _(gp):
            gp.load_library(_mlp_lib)
            gp.dma_start(xwt[:], xw_d.ap()[:]).then_inc(io, 16)
            gp.dma_start(xt[:], x_d.ap()[:]).then_inc(io, 16)
            for r in range(reps):
                gp.wait_ge(cs, 2 * r + 1)
                # swdge FIFO holds 128 packed entries (~32 idxs each), so the
                # 32768-row gather goes out as chunks; entries reclaim as the
                # self-triggered DMAs drain.
                for k in range(NCHUNK):
                    gp.dma_gather(gv[:, k * CH_COLS : (k + 1) * CH_COLS, :],
                                  tab_ds[k % 4].ap()[:],
                                  idxt[:, k * CH_W : (k + 1) * CH_W],
                                  CH_IDX, CH_IDX, ELEM, single_packet=False,
                                  queue_num=k % 4).then_inc(gs, 16)
            gp.wait_ge(cs, 2 * reps)
            gp.dma_start(y_d.ap()[:], yt[:]).then_inc(io, 16)
            gp.wait_ge(io, 48)

        @block.vector
        def _(v):
            # DVE completion is async even in program order: chain every
            # dependent instruction on a per-engine counting semaphore
            # (same idiom the Tile scheduler emits).
            cnt = [0]

            def chain(ins):
                ins.then_inc(vs, 1)
                cnt[0] += 1
                return ins

            v.wait_ge(io, 32)
            for r in range(reps):
                if cnt[0]:
                    v.wait_ge(vs, cnt[0])
                # interval index in the wrapped layout, as int16
                chain(v.tensor_scalar(out=zwt[:], in0=xwt[:], scalar1=_INV_H,
                                      scalar2=_CB, op0=AL.mult, op1=AL.add))
                v.wait_ge(vs, cnt[0])
                chain(v.tensor_scalar(out=zwt[:], in0=zwt[:], scalar1=0.0,
                                      scalar2=_CLMAX, op0=AL.max, op1=AL.min))
                v.wait_ge(vs, cnt[0])
                # milestone instructions update cs instead of vs (one sem
                # update per instruction); ordering of later DVE work is
                # carried transitively through cs -> gather -> gs.
                v.tensor_scalar(out=idxt[:], in0=zwt[:], scalar1=0.5,
                                scalar2=None, op0=AL.subtract).then_inc(cs, 1)
                v.wait_ge(gs, 16 * NCHUNK * (r + 1))
                # u = x - m_i; Horner; mask to zero outside [T0, TLAST)
                chain(v.tensor_tensor(out=ut[:], in0=xt[:], in1=col(4),
                                      op=AL.subtract))
                v.wait_ge(vs, cnt[0])
                chain(v.tensor_tensor(out=acct[:], in0=col(3), in1=ut[:],
                                      op=AL.mult))
                v.wait_ge(vs, cnt[0])
                chain(v.tensor_tensor(out=acct[:], in0=acct[:], in1=col(2),
                                      op=AL.add))
                v.wait_ge(vs, cnt[0])
                chain(v.tensor_tensor(out=acct[:], in0=acct[:], in1=ut[:],
                                      op=AL.mult))
                v.wait_ge(vs, cnt[0])
                chain(v.tensor_tensor(out=acct[:], in0=acct[:], in1=col(1),
                                      op=AL.add))
                v.wait_ge(vs, cnt[0])
                chain(v.tensor_tensor(out=acct[:], in0=acct[:], in1=ut[:],
                                      op=AL.mult))
                v.wait_ge(vs, cnt[0])
                chain(v.tensor_tensor(out=acct[:], in0=acct[:], in1=col(0),
                                      op=AL.add))
                v.wait_ge(vs, cnt[0])
                chain(v.scalar_tensor_tensor(out=m1t[:], in0=xt[:],
                                             scalar=_TLAST, in1=acct[:],
                                             op0=AL.is_lt, op1=AL.mult))
                v.wait_ge(vs, cnt[0])
                v.scalar_tensor_tensor(out=yt[:], in0=xt[:], scalar=_T0,
                                       in1=m1t[:], op0=AL.is_ge,
                                       op1=AL.mult).then_inc(cs, 1)

    nc.compile()
    return nc


def _in_maps(x, weights):
    tab = _build_table(np.asarray(weights))
    xs = np.asarray(x, dtype=np.float32).reshape(NCORES, NPTS_C)
    maps = []
    for c in range(NCORES):
        xc = xs[c]
        xd = np.ascontiguousarray(xc.reshape(F, P).T)            # [128, 256]
        xw = np.ascontiguousarray(
            np.tile(xc.reshape(WRAP_F, 16).T, (NCORES, 1)))      # [128, 2048]
        maps.append({"x": xd, "xw": xw, "tab0": tab, "tab1": tab,
                     "tab2": tab, "tab3": tab})
    return maps


def kernel(x: np.ndarray, weights: np.ndarray) -> np.ndarray:
    if "nc" not in _NC_CACHE:
        _NC_CACHE["nc"] = _build_nc()
    nc = _NC_CACHE["nc"]
    res = run_bass_kernel_spmd(nc, _in_maps(x, weights),
                               core_ids=list(range(NCORES)))
    y = np.stack([res.results[c]["y"].T.ravel() for c in range(NCORES)], 0)
    return y.reshape(NPTS, 1).astype(np.float32)


def estimate_hw_ns(x=None, weights=None, reps_hi: int = 25,
                   timing_reps: int = 12) -> int:
    """Device time per kernel body: wall-clock delta between reps=1 and
    reps=reps_hi builds (amplification cancels host/launch overhead)."""
    import time as _time

    if x is None:
        rng = np.random.default_rng(0)
        x = rng.standard_normal((NPTS, 1)).astype(np.float32)
        weights = rng.standard_normal((NW,)).astype(np.float32)
    im = _in_maps(x, weights)
    walls = {}
    for reps in (1, reps_hi):
        nc = _NC_CACHE.get(("nc", reps))
        if nc is None:
            nc = _build_nc(reps) if reps > 1 else _NC_CACHE.get("nc") or _build_nc()
            _NC_CACHE[("nc", reps)] = nc
        run_bass_kernel_spmd(nc, im, core_ids=list(range(NCORES)))
        ts = []
        for _ in range(timing_reps):
            t0 = _time.perf_counter()
            run_bass_kernel_spmd(nc, im, core_ids=list(range(NCORES)))
            ts.append(_time.perf_counter() - t0)
        walls[reps] = min(ts)
    return int((walls[reps_hi] - walls[1]) / (reps_hi - 1) * 1e9)
